# revision 16
# baseline (speedup 1.0000x reference)
"""Causal self-attention on 8 TRN2 NeuronCores.

Problem: x[2,2048,1024], wq/wk/wv/wo[1024,1024] (nn.Linear convention,
out = y @ W.T), H=16 heads, D=64, causal softmax, f32.

Sharding: tensor-parallel over heads x data-parallel over batch.
Core i handles batch b=i//4 and head group g=i%4 (4 heads each).
wq/wk/wv are split row-wise (output-feature) per head group; wo is
split column-wise; each core returns a partial output projection
out_partial[b] and the host sums the 4 partials per batch.

On-device layout is fully "feature-major" (transposed): the host passes
xT=x[b].T etc so every matmul sees its contraction dim on SBUF
partitions and no on-device transposes are needed.

Attention processes heads in PAIRS with block-diagonal stationary
operands so every matmul lights the full 128x128 PE array (TRN2's
activity throttler clocks the PE down to 1.2GHz whenever array
utilization sits near 50%, which is what per-head K=64 scores and
M=65 PV matmuls produce):
  scores^T: lhsT = [[kT_a,0],[0,kT_b]] (per 64-wide key-position
  half), rhs = the naturally head-stacked qT tile -> psum rows 0:64 =
  S^T_a, 64:128 = S^T_b.  exp runs on ScalarE in wide strokes over
  two key-halves at once; causal masking is a multiplicative bf16
  mask on P^T after exp.  PV: lhsT = [[V_a,0],[0,V_b]] against the
  stacked P^T -> Y^T for both heads in one matmul; the softmax row
  sums come from a [[J,0],[0,J]] ones-block stationary (replicated
  rows, full-array).  The 1/sum normalization transposes the sums via
  a permuted DRAM-roundtrip DMA so DVE reciprocal runs on 128
  partitions instead of one.  Projections and scores run in float32r
  (TF32-like, 1 cycle/row at N>=256); P^T and V in bf16 with fp32
  PSUM accumulation.
"""

import sys

for _p in ("/opt/trn_rl_repo", "/root/.axon_site"):
    if _p not in sys.path:
        sys.path.insert(0, _p)

import numpy as np

import concourse.bass as bass
import concourse.mybir as mybir
import concourse.tile as tile
from concourse import bacc
from concourse.bass_utils import run_bass_kernel_spmd

B, T, C, H = 2, 2048, 1024, 16
DH = C // H            # 64 head dim
HG = 4                 # heads per core
GW = HG * DH           # 256 features per head group
NKH = T // 64          # 32 key-position halves
NS = T // 512          # 4 query spans
KC = C // 128          # 8 contraction chunks over C
SCALE = 1.0 / float(np.sqrt(DH))
N_CORES = 8

F32 = mybir.dt.float32
F32R = mybir.dt.float32r
BF16 = mybir.dt.bfloat16
EXP = mybir.ActivationFunctionType.Exp
COPY = mybir.ActivationFunctionType.Copy


def build_nc():
    nc = bacc.Bacc("TRN2", target_bir_lowering=False, debug=False,
                   num_devices=N_CORES)
    xT = nc.declare_dram_parameter("xT", [C, T], F32R, isOutput=False)
    wqT = nc.declare_dram_parameter("wqT", [C, GW], F32R, isOutput=False)
    wkT = nc.declare_dram_parameter("wkT", [C, GW], F32R, isOutput=False)
    wvT = nc.declare_dram_parameter("wvT", [C, GW], F32R, isOutput=False)
    woT = nc.declare_dram_parameter("woT", [GW, C], F32R, isOutput=False)
    outT = nc.declare_dram_parameter("outT", [C, T], F32, isOutput=True)
    s_dram = nc.dram_tensor("s_scratch", [2, NS, 2, 512], F32)
    r_dram = nc.dram_tensor("r_scratch", [2, NS, 2, 512], F32)

    with tile.TileContext(nc) as tc:
        with tc.tile_pool(name="pers", bufs=1) as pers:
            # ---- persistent tensors that live to the end ----
            wo_t = [pers.tile([128, C], F32R, tag=f"wo{j}", name=f"wo{j}")
                    for j in range(2)]
            qts = [pers.tile([128, T], F32R, tag=f"qT{m}", name=f"qT{m}")
                   for m in range(2)]
            yts = [pers.tile([128, T], F32R, tag=f"yT{m}", name=f"yT{m}")
                   for m in range(2)]
            # block-diagonal stationaries per (pair, key-half)
            kds = [[pers.tile([128, 128], F32R, tag=f"kd{p}_{k}",
                              name=f"kd{p}_{k}") for k in range(NKH)]
                   for p in range(2)]
            vds = [[pers.tile([128, 128], BF16, tag=f"vd{p}_{k}",
                              name=f"vd{p}_{k}") for k in range(NKH)]
                   for p in range(2)]
            onesJ = pers.tile([128, 128], BF16, tag="onesJ", name="onesJ")
            nc.gpsimd.memset(onesJ, 0.0)
            nc.gpsimd.memset(onesJ[0:64, 0:64], 1.0)
            nc.gpsimd.memset(onesJ[64:128, 64:128], 1.0)
            zero_f32 = pers.tile([128, 128], F32, tag="zf", name="zf")
            nc.gpsimd.memset(zero_f32, 0.0)
            # diagonal masks per z = (key-half - 8s): cols [0,64z)=0,
            # tri at [64z,64z+64) keeping col-64z >= row%64, then 1
            dms = []
            for z in range(8):
                dm = pers.tile([128, 512], BF16, tag=f"dm{z}", name=f"dm{z}")
                nc.gpsimd.memset(dm, 1.0)
                if z:
                    nc.gpsimd.memset(dm[:, 0:64 * z], 0.0)
                for hf in range(2):
                    nc.gpsimd.affine_select(
                        out=dm[64 * hf:64 * hf + 64, 64 * z:64 * z + 64],
                        in_=dm[64 * hf:64 * hf + 64, 64 * z:64 * z + 64],
                        compare_op=mybir.AluOpType.is_ge,
                        fill=0.0, base=0, pattern=[[1, 64]],
                        channel_multiplier=-1)
                dms.append(dm)

            # ---- phase 1: load + projections (inputs freed after) ----
            with tc.tile_pool(name="ld", bufs=1) as ld, \
                 tc.tile_pool(name="pp1", bufs=6, space="PSUM") as pp1, \
                 tc.tile_pool(name="pp2", bufs=2, space="PSUM") as pp2:
                wk_t = [ld.tile([128, GW], F32R, tag=f"wk{i}", name=f"wk{i}")
                        for i in range(KC)]
                for i in range(KC):
                    nc.sync.dma_start(out=wk_t[i],
                                      in_=wkT[i * 128:(i + 1) * 128, :])
                xts = [ld.tile([128, T], F32R, tag=f"xT{i}", name=f"xT{i}")
                       for i in range(KC)]
                wq_t = [ld.tile([128, GW], F32R, tag=f"wq{i}", name=f"wq{i}")
                        for i in range(KC)]
                for s in range(NS):      # span-sized sub-loads
                    for i in range(KC):
                        nc.sync.dma_start(
                            out=xts[i][:, s * 512:(s + 1) * 512],
                            in_=xT[i * 128:(i + 1) * 128,
                                   s * 512:(s + 1) * 512])
                    if s == 0:
                        for i in range(KC):
                            nc.sync.dma_start(
                                out=wq_t[i],
                                in_=wqT[i * 128:(i + 1) * 128, :])
                wv_t = [ld.tile([128, GW], F32R, tag=f"wv{i}", name=f"wv{i}")
                        for i in range(KC)]
                for i in range(KC):
                    nc.sync.dma_start(out=wv_t[i],
                                      in_=wvT[i * 128:(i + 1) * 128, :])
                for j in range(2):
                    nc.sync.dma_start(out=wo_t[j],
                                      in_=woT[j * 128:(j + 1) * 128, :])

                # zero-fill the block-diagonal stationaries (DVE cast)
                for p in range(2):
                    for k in range(NKH):
                        nc.vector.tensor_copy(out=kds[p][k], in_=zero_f32)
                        nc.vector.memset(vds[p][k], 0.0)

                # kT: psum block (pair p, span s) is [128 d-stacked,
                # 512 kpos]; scatter 64-wide kpos slices into the
                # diagonal blocks of kds
                for wt, isq in ((wk_t, False), (wq_t, True)):
                    for p in range(2):
                        for s in range(NS):
                            ps = pp1.tile([128, 512], F32, tag="projps",
                                          name="projps")
                            for k in range(KC):
                                nc.tensor.matmul(
                                    ps,
                                    wt[k][:, p * 128:(p + 1) * 128],
                                    xts[k][:, s * 512:(s + 1) * 512],
                                    start=(k == 0), stop=(k == KC - 1))
                            if isq:
                                nc.vector.tensor_copy(
                                    out=qts[p][:, s * 512:(s + 1) * 512],
                                    in_=ps)
                            else:
                                for z in range(8):
                                    kd = kds[p][8 * s + z]
                                    nc.vector.tensor_copy(
                                        out=kd[0:64, 0:64],
                                        in_=ps[0:64, 64 * z:64 * z + 64])
                                    nc.vector.tensor_copy(
                                        out=kd[64:128, 64:128],
                                        in_=ps[64:128, 64 * z:64 * z + 64])
                # V: psum block tb is [128 t, 256 d(4 heads)]; scatter
                # 64-row kpos halves into vds diagonal blocks
                for tb in range(T // 128):
                    vps = pp2.tile([128, GW], F32, tag="vps", name="vps")
                    for k in range(KC):
                        nc.tensor.matmul(
                            vps, xts[k][:, tb * 128:(tb + 1) * 128], wv_t[k],
                            start=(k == 0), stop=(k == KC - 1))
                    for p in range(2):
                        for o in range(2):
                            vd = vds[p][2 * tb + o]
                            nc.vector.tensor_copy(
                                out=vd[0:64, 0:64],
                                in_=vps[64 * o:64 * o + 64,
                                        128 * p:128 * p + 64])
                            nc.vector.tensor_copy(
                                out=vd[64:128, 64:128],
                                in_=vps[64 * o:64 * o + 64,
                                        128 * p + 64:128 * p + 128])

            # ---- phase 2: attention, head-pair x span, key-half inner ----
            with tc.tile_pool(name="mgs", bufs=2, space="PSUM") as mgs, \
                 tc.tile_pool(name="pvs", bufs=2, space="PSUM") as pvs, \
                 tc.tile_pool(name="sms", bufs=2, space="PSUM") as sms, \
                 tc.tile_pool(name="ptp", bufs=5) as ptp, \
                 tc.tile_pool(name="rp", bufs=3) as rp:
                for p in range(2):
                    qt, yt = qts[p], yts[p]
                    for s in range(NS):
                        nkh = 8 * s + 8
                        pvt = pvs.tile([128, 512], F32, tag="pvt", name="pvt")
                        smt = sms.tile([128, 512], F32, tag="smt", name="smt")
                        for kh0 in range(0, nkh, 2):
                            ces = []
                            mg = mgs.tile([128, 1024], F32, tag="mg",
                                          name="mg")
                            for u in range(2):
                                kh = kh0 + u
                                z = kh - 8 * s
                                ce = min(64 * z, 256) if z >= 0 else 0
                                ces.append((kh, z, ce))
                                nc.tensor.matmul(
                                    mg[:, 512 * u + ce:512 * (u + 1)],
                                    kds[p][kh],
                                    qt[:, 512 * s + ce:512 * (s + 1)],
                                    start=True, stop=True)
                            pt = ptp.tile([128, 1024], BF16, tag="pt",
                                          name="pt")
                            ce0, ce1 = ces[0][2], ces[1][2]
                            if ce1 == 0:
                                nc.scalar.activation(
                                    out=pt[:, ce0:1024], in_=mg[:, ce0:1024],
                                    func=EXP, scale=SCALE)
                            else:
                                nc.scalar.activation(
                                    out=pt[:, ce0:512], in_=mg[:, ce0:512],
                                    func=EXP, scale=SCALE)
                                nc.scalar.activation(
                                    out=pt[:, 512 + ce1:1024],
                                    in_=mg[:, 512 + ce1:1024],
                                    func=EXP, scale=SCALE)
                            for u in range(2):
                                kh, z, ce = ces[u]
                                if z >= 0:
                                    nc.vector.tensor_mul(
                                        out=pt[:, 512 * u + ce:512 * (u + 1)],
                                        in0=pt[:, 512 * u + ce:512 * (u + 1)],
                                        in1=dms[z][:, ce:512])
                                nc.tensor.matmul(
                                    pvt[:, ce:],
                                    vds[p][kh],
                                    pt[:, 512 * u + ce:512 * (u + 1)],
                                    start=(kh == 0), stop=(kh == nkh - 1))
                                nc.tensor.matmul(
                                    smt[:, ce:],
                                    onesJ,
                                    pt[:, 512 * u + ce:512 * (u + 1)],
                                    start=(kh == 0), stop=(kh == nkh - 1))
                        # finalize: free banks fast, then normalize via
                        # a DMA-transposed reciprocal off the hot path
                        yv = rp.tile([128, 512], F32, tag="yv", name="yv")
                        nc.vector.tensor_copy(out=yv, in_=pvt)
                        ssa = rp.tile([1, 512], F32, tag="ssa", name="ssa")
                        nc.vector.tensor_copy(out=ssa, in_=smt[0:1, :])
                        ssb = rp.tile([1, 512], F32, tag="ssb", name="ssb")
                        nc.vector.tensor_copy(out=ssb, in_=smt[64:65, :])
                        nc.gpsimd.dma_start(out=s_dram[p, s, 0], in_=ssa)
                        nc.gpsimd.dma_start(out=s_dram[p, s, 1], in_=ssb)
                        st = rp.tile([128, 8], F32, tag="st", name="st")
                        for hf in range(2):
                            nc.gpsimd.dma_start(
                                out=st[:, 4 * hf:4 * hf + 4],
                                in_=s_dram[p, s, hf, :].rearrange(
                                    "(c q) -> q c", q=128))
                        rts = rp.tile([128, 8], F32, tag="rts", name="rts")
                        nc.vector.reciprocal(out=rts, in_=st)
                        for hf in range(2):
                            nc.gpsimd.dma_start(
                                out=r_dram[p, s, hf, :].rearrange(
                                    "(c q) -> q c", q=128),
                                in_=rts[:, 4 * hf:4 * hf + 4])
                        rb = rp.tile([128, 512], F32, tag="rb", name="rb")
                        for hf in range(2):
                            rsl = r_dram[p, s, hf, :]
                            nc.gpsimd.dma_start(
                                out=rb[64 * hf:64 * hf + 64, :],
                                in_=bass.AP(tensor=rsl.tensor,
                                            offset=rsl.offset,
                                            ap=[[0, 64]] + list(rsl.ap)))
                        nc.vector.tensor_mul(
                            out=yt[:, 512 * s:512 * (s + 1)],
                            in0=yv, in1=rb)

            # ---- phase 3: output projection (partial sums) ----
            with tc.tile_pool(name="ops", bufs=4, space="PSUM") as ops, \
                 tc.tile_pool(name="ost", bufs=4) as ost:
                for m in range(8):
                    for s in range(NS):
                        op = ops.tile([128, 512], F32, tag="op", name="op")
                        for j in range(2):
                            nc.tensor.matmul(
                                op,
                                wo_t[j][:, m * 128:(m + 1) * 128],
                                yts[j][:, s * 512:(s + 1) * 512],
                                start=(j == 0), stop=(j == 1))
                        ot = ost.tile([128, 512], F32, tag="ot", name="ot")
                        nc.vector.tensor_copy(out=ot, in_=op)
                        nc.sync.dma_start(
                            out=outT[m * 128:(m + 1) * 128,
                                     s * 512:(s + 1) * 512],
                            in_=ot)
    nc.compile()
    return nc


_NC_CACHE = None


def _get_nc():
    global _NC_CACHE
    if _NC_CACHE is None:
        _NC_CACHE = build_nc()
    return _NC_CACHE


def make_in_maps(x, wq, wk, wv, wo):
    x = np.asarray(x, dtype=np.float32)
    wq = np.asarray(wq, dtype=np.float32)
    wk = np.asarray(wk, dtype=np.float32)
    wv = np.asarray(wv, dtype=np.float32)
    wo = np.asarray(wo, dtype=np.float32)
    in_maps = []
    for core in range(N_CORES):
        b, g = core // HG, core % HG
        rows = slice(g * GW, (g + 1) * GW)
        in_maps.append({
            "xT": np.ascontiguousarray(x[b].T),
            "wqT": np.ascontiguousarray(wq[rows, :].T),
            "wkT": np.ascontiguousarray(wk[rows, :].T),
            "wvT": np.ascontiguousarray(wv[rows, :].T),
            "woT": np.ascontiguousarray(wo[:, rows].T),
        })
    return in_maps


def run(x, wq, wk, wv, wo, trace=False, tmpdir=None):
    nc = _get_nc()
    in_maps = make_in_maps(x, wq, wk, wv, wo)
    res = run_bass_kernel_spmd(nc, in_maps, core_ids=list(range(N_CORES)),
                               trace=trace, tmpdir=tmpdir)
    out = np.zeros((B, T, C), dtype=np.float32)
    for core in range(N_CORES):
        out[core // HG] += res.results[core]["outT"].T
    return out, res


def kernel(x, wq, wk, wv, wo):
    out, _ = run(x, wq, wk, wv, wo)
    return out
